# revision 1
# baseline (speedup 1.0000x reference)
"""FWHT (normalized Walsh-Hadamard transform) over the last dim of x[4,4096,4096].

Strategy: rows are independent -> shard 16384 rows across 8 NeuronCores
(2048 rows each).  Per row, H_4096 = H_32 (x) H_128 (Sylvester Kronecker
factorization), so with the row viewed as X[c, kc] (c in [0,32), kc in [0,128)):

    y[c'*128+kc'] = (1/64) * sum_{c,kc} H32[c',c] H128[kc',kc] X[c,kc]

Per 256-row block on a core:
  T0:  TensorE transposes [r | k-chunk] -> Z[kc | r] per c-chunk
  A :  matmul, stationary H128/8, moving Z_c            -> B[kc' | r] per c
  T1:  TensorE transposes gather (rr,c) slices of B     -> Z2[(rr,c) | kc']
  B :  matmul, stationary kron(I4,H32)/8, moving Z2     -> C[(rr',c') | (u,kc')]
  DMA out with 512B-contiguous runs (kc' fastest).
"""

import numpy as np

try:
    import concourse.bass as bass  # noqa: F401
except ImportError:
    import sys

    sys.path.insert(0, "/opt/trn_rl_repo")

from concourse import bacc, bass, bass_utils, tile
from concourse import mybir

F32 = mybir.dt.float32
F32R = mybir.dt.float32r

N_CORES = 8
ROWS_PER_CORE = 2048
DIM = 4096
N_BLOCKS = 8       # blocks of 256 rows per core
BLOCK_ROWS = 256


def _hadamard(n: int) -> np.ndarray:
    h = np.array([[1.0]], dtype=np.float64)
    while h.shape[0] < n:
        h = np.block([[h, h], [h, -h]])
    return h


def _constants():
    h128 = (_hadamard(128) * 0.125).astype(np.float32)
    m32 = (np.kron(np.eye(4), _hadamard(32)) * 0.125).astype(np.float32)
    ident = np.eye(128, dtype=np.float32)
    return h128, m32, ident


def build_program(repeat: int = 1, hw_loop: bool = False,
                  dummy_out_dma: bool = False):
    nc = bacc.Bacc(
        "TRN2",
        target_bir_lowering=False,
        debug=False,
        enable_asserts=False,
    )

    x_d = nc.dram_tensor("x", [ROWS_PER_CORE, DIM], F32R, kind="ExternalInput").ap()
    h128_d = nc.dram_tensor("h128", [128, 128], F32R, kind="ExternalInput").ap()
    m32_d = nc.dram_tensor("m32", [128, 128], F32R, kind="ExternalInput").ap()
    id_d = nc.dram_tensor("ident", [128, 128], F32R, kind="ExternalInput").ap()
    y_d = nc.dram_tensor("y", [ROWS_PER_CORE, DIM], F32, kind="ExternalOutput").ap()

    with tile.TileContext(nc) as tc:
        with (
            tc.tile_pool(name="const", bufs=1) as const_pool,
            tc.tile_pool(name="xin", bufs=3) as x_pool,
            tc.tile_pool(name="zbuf", bufs=1) as z_pool,
            tc.tile_pool(name="bbuf", bufs=1) as b_pool,
            tc.tile_pool(name="z2buf", bufs=4) as z2_pool,
            tc.tile_pool(name="obuf", bufs=3) as o_pool,
            tc.tile_pool(name="ps_t0", bufs=2, space="PSUM") as t0_psum,
            tc.tile_pool(name="ps_a", bufs=2, space="PSUM") as a_psum,
            tc.tile_pool(name="ps_t1", bufs=2, space="PSUM") as t1_psum,
            tc.tile_pool(name="ps_b", bufs=2, space="PSUM") as b_psum,
        ):
            h128_t = const_pool.tile([128, 128], F32R)
            m32_t = const_pool.tile([128, 128], F32R)
            id_t = const_pool.tile([128, 128], F32R)
            nc.sync.dma_start(h128_t[:], h128_d)
            nc.sync.dma_start(m32_t[:], m32_d)
            nc.sync.dma_start(id_t[:], id_d)
            h128_r = h128_t[:]
            m32_r = m32_t[:]

            import contextlib

            loop_ctx = (
                tc.For_i(0, repeat) if hw_loop and repeat > 1
                else contextlib.nullcontext()
            )
            with loop_ctx:
              for b in range(N_BLOCKS * (1 if hw_loop else repeat)):
                  r0 = (b % N_BLOCKS) * BLOCK_ROWS

                  # ---- load 2 x [128, 4096] row subtiles ----
                  xt = []
                  for rs in range(2):
                      t = x_pool.tile([128, DIM], F32R)
                      eng = nc.sync if rs == 0 else nc.scalar
                      eng.dma_start(
                          t[:], x_d[r0 + rs * 128 : r0 + (rs + 1) * 128, :]
                      )
                      xt.append(t)

                  # ---- T0: Z[kc, c*256 + r_local], r_local = rs*128 + i ----
                  z = z_pool.tile([128, 32 * 256], F32R)
                  for cp in range(16):
                      ps = t0_psum.tile([128, 512], F32R)
                      for k in range(2):
                          c = cp * 2 + k
                          for rs in range(2):
                              nc.tensor.transpose(
                                  ps[:, k * 256 + rs * 128 : k * 256 + (rs + 1) * 128],
                                  xt[rs][:, c * 128 : (c + 1) * 128],
                                  id_t[:],
                              )
                      # split copies between ACT and DVE for load balance
                      if cp % 2 == 0:
                          nc.scalar.copy(z[:, cp * 512 : (cp + 1) * 512], ps[:])
                      else:
                          nc.vector.tensor_copy(z[:, cp * 512 : (cp + 1) * 512], ps[:])

                  # ---- stage A: B[kc', r_local*32 + c] (scatter per-c so T1
                  # reads contiguous 128-slices; PE rhs allows only 1 free dim)
                  bb = b_pool.tile([128, 32 * 256], F32R)
                  bb_sc = bb[:].rearrange("p (r c) -> p c r", c=32)
                  for cp in range(16):
                      ps = a_psum.tile([128, 512], F32)
                      nc.tensor.matmul(
                          ps[:], h128_r, z[:, cp * 512 : (cp + 1) * 512]
                      )
                      for k in range(2):
                          c = cp * 2 + k
                          if cp % 2 == 0:
                              nc.scalar.copy(
                                  bb_sc[:, c], ps[:, k * 256 : (k + 1) * 256]
                              )
                          else:
                              nc.vector.tensor_copy(
                                  bb_sc[:, c], ps[:, k * 256 : (k + 1) * 256]
                              )

                  out_halves = [
                      o_pool.tile(
                          [128, 16 * 256], F32, name=f"out_half_{b}_{i}", tag="out_half"
                      )
                      for i in range(2)
                  ]
                  for wp in range(16):
                      out = out_halves[wp // 8]
                      wo = wp % 8  # wp within half
                      # ---- T1: Z2[(rr,c), (w2,u,kc')] for w pair ----
                      ps = t1_psum.tile([128, 512], F32R)
                      for k in range(2):
                          w = wp * 2 + k
                          for u in range(2):
                              f0 = (w * 8 + u * 4) * 32
                              nc.tensor.transpose(
                                  ps[:, k * 256 + u * 128 : k * 256 + (u + 1) * 128],
                                  bb[:, f0 : f0 + 128],
                                  id_t[:],
                              )
                      z2 = z2_pool.tile([128, 512], F32R)
                      nc.scalar.copy(z2[:], ps[:])

                      # ---- stage B ----
                      psb = b_psum.tile([128, 512], F32)
                      nc.tensor.matmul(psb[:], m32_r, z2[:])
                      nc.vector.tensor_copy(out[:, wo * 512 : (wo + 1) * 512], psb[:])

                      # ---- DMA out after each half: [(rr',c'), (w,u,kc')] -> y ----
                      if wo == 7:
                          h = wp // 8
                          yb = y_d[r0 : r0 + BLOCK_ROWS, :].rearrange(
                              "(w u rr) (cp kc) -> rr cp w u kc", w=32, u=2, rr=4, cp=32
                          )
                          ob = out[:].rearrange("p (w u kc) -> p w u kc", w=16, u=2)
                          w0 = h * 16
                          nc.sync.dma_start(
                              yb[:, :, w0 : w0 + 8], ob[:, 0:8]
                          )
                          nc.scalar.dma_start(
                              yb[:, :, w0 + 8 : w0 + 16], ob[:, 8:16]
                          )

    nc.compile()
    return nc


_CACHE = {}


def _get_program():
    if "nc" not in _CACHE:
        _CACHE["nc"] = build_program()
    return _CACHE["nc"]


def kernel(x: np.ndarray, _trace: bool = False, _trace_kwargs=None) -> np.ndarray:
    assert x.shape == (4, 4096, 4096), x.shape
    x_flat = np.ascontiguousarray(x.reshape(16384, DIM), dtype=np.float32)
    h128, m32, ident = _constants()

    in_maps = []
    for i in range(N_CORES):
        in_maps.append(
            {
                "x": x_flat[i * ROWS_PER_CORE : (i + 1) * ROWS_PER_CORE],
                "h128": h128,
                "m32": m32,
                "ident": ident,
            }
        )

    nc = _get_program()
    res = bass_utils.run_bass_kernel_spmd(
        nc,
        in_maps,
        core_ids=list(range(N_CORES)),
        trace=_trace,
        **(_trace_kwargs or {}),
    )
    outs = [res.results[i]["y"] for i in range(N_CORES)]
    y = np.concatenate(outs, axis=0).reshape(4, 4096, 4096)
    if _trace:
        _CACHE["last_result"] = res
    return y



# revision 2
# speedup vs baseline: 5.3607x; 5.3607x over previous
"""FWHT (normalized Walsh-Hadamard transform) over the last dim of x[4,4096,4096].

Strategy: rows are independent -> shard 16384 rows across 8 NeuronCores
(2048 rows each).  Per row, H_4096 = H_32 (x) H_128 (Sylvester Kronecker
factorization), so with the row viewed as X[c, kc] (c in [0,32), kc in [0,128)):

    y[c'*128+kc'] = (1/64) * sum_{c,kc} H32[c',c] H128[kc',kc] X[c,kc]

Per 256-row block on a core:
  T0:  TensorE transposes [r | k-chunk] -> Z[kc | r] per c-chunk
  A :  matmul, stationary H128/8, moving Z_c            -> B[kc' | r] per c
  T1:  TensorE transposes gather (rr,c) slices of B     -> Z2[(rr,c) | kc']
  B :  matmul, stationary kron(I4,H32)/8, moving Z2     -> C[(rr',c') | (u,kc')]
  DMA out with 512B-contiguous runs (kc' fastest).
"""

import numpy as np

try:
    import concourse.bass as bass  # noqa: F401
except ImportError:
    import sys

    sys.path.insert(0, "/opt/trn_rl_repo")

from concourse import bacc, bass, bass_utils, tile
from concourse import mybir

F32 = mybir.dt.float32
F32R = mybir.dt.float32r

N_CORES = 8
ROWS_PER_CORE = 2048
DIM = 4096
N_BLOCKS = 8       # blocks of 256 rows per core
BLOCK_ROWS = 256


def _hadamard(n: int) -> np.ndarray:
    h = np.array([[1.0]], dtype=np.float64)
    while h.shape[0] < n:
        h = np.block([[h, h], [h, -h]])
    return h


def _constants():
    h128 = (_hadamard(128) * 0.125).astype(np.float32)
    m32 = (np.kron(np.eye(4), _hadamard(32)) * 0.125).astype(np.float32)
    ident = np.eye(128, dtype=np.float32)
    return h128, m32, ident


def build_program(repeat: int = 1, hw_loop: bool = False,
                  dummy_out_dma: bool = False):
    nc = bacc.Bacc(
        "TRN2",
        target_bir_lowering=False,
        debug=False,
        enable_asserts=False,
    )

    x_d = nc.dram_tensor("x", [ROWS_PER_CORE, DIM], F32R, kind="ExternalInput").ap()
    h128_d = nc.dram_tensor("h128", [128, 128], F32R, kind="ExternalInput").ap()
    m32_d = nc.dram_tensor("m32", [128, 128], F32R, kind="ExternalInput").ap()
    id_d = nc.dram_tensor("ident", [128, 128], F32R, kind="ExternalInput").ap()
    y_d = nc.dram_tensor("y", [ROWS_PER_CORE, DIM], F32, kind="ExternalOutput").ap()

    with tile.TileContext(nc) as tc:
        with (
            tc.tile_pool(name="const", bufs=1) as const_pool,
            tc.tile_pool(name="xin", bufs=3) as x_pool,
            tc.tile_pool(name="zbuf", bufs=1) as z_pool,
            tc.tile_pool(name="bbuf", bufs=1) as b_pool,
            tc.tile_pool(name="z2buf", bufs=4) as z2_pool,
            tc.tile_pool(name="obuf", bufs=3) as o_pool,
            tc.tile_pool(name="ps_t0", bufs=2, space="PSUM") as t0_psum,
            tc.tile_pool(name="ps_a", bufs=2, space="PSUM") as a_psum,
            tc.tile_pool(name="ps_t1", bufs=2, space="PSUM") as t1_psum,
            tc.tile_pool(name="ps_b", bufs=2, space="PSUM") as b_psum,
        ):
            h128_t = const_pool.tile([128, 128], F32R)
            m32_t = const_pool.tile([128, 128], F32R)
            id_t = const_pool.tile([128, 128], F32R)
            nc.sync.dma_start(h128_t[:], h128_d)
            nc.sync.dma_start(m32_t[:], m32_d)
            nc.sync.dma_start(id_t[:], id_d)
            h128_r = h128_t[:]
            m32_r = m32_t[:]

            import contextlib

            loop_ctx = (
                tc.For_i(0, repeat) if hw_loop and repeat > 1
                else contextlib.nullcontext()
            )
            with loop_ctx:
              for b in range(N_BLOCKS * (1 if hw_loop else repeat)):
                  r0 = (b % N_BLOCKS) * BLOCK_ROWS

                  # ---- load 2 x [128, 4096] row subtiles ----
                  xt = []
                  for rs in range(2):
                      t = x_pool.tile([128, DIM], F32R)
                      eng = nc.sync if rs == 0 else nc.scalar
                      eng.dma_start(
                          t[:], x_d[r0 + rs * 128 : r0 + (rs + 1) * 128, :]
                      )
                      xt.append(t)

                  # ---- T0: Z[kc, c*256 + r_local], r_local = rs*128 + i ----
                  z = z_pool.tile([128, 32 * 256], F32R)
                  for cp in range(16):
                      ps = t0_psum.tile([128, 512], F32R)
                      for k in range(2):
                          c = cp * 2 + k
                          for rs in range(2):
                              nc.tensor.transpose(
                                  ps[:, k * 256 + rs * 128 : k * 256 + (rs + 1) * 128],
                                  xt[rs][:, c * 128 : (c + 1) * 128],
                                  id_t[:],
                              )
                      # split copies between ACT and DVE for load balance
                      if cp % 2 == 0:
                          nc.scalar.copy(z[:, cp * 512 : (cp + 1) * 512], ps[:])
                      else:
                          nc.vector.tensor_copy(z[:, cp * 512 : (cp + 1) * 512], ps[:])

                  # ---- stage A: B[kc', r_local*32 + c] (scatter per-c so T1
                  # reads contiguous 128-slices; PE rhs allows only 1 free dim)
                  bb = b_pool.tile([128, 32 * 256], F32R)
                  bb_sc = bb[:].rearrange("p (r c) -> p c r", c=32)
                  for cp in range(16):
                      ps = a_psum.tile([128, 512], F32)
                      nc.tensor.matmul(
                          ps[:], h128_r, z[:, cp * 512 : (cp + 1) * 512]
                      )
                      for k in range(2):
                          c = cp * 2 + k
                          if cp % 2 == 0:
                              nc.scalar.copy(
                                  bb_sc[:, c], ps[:, k * 256 : (k + 1) * 256]
                              )
                          else:
                              nc.vector.tensor_copy(
                                  bb_sc[:, c], ps[:, k * 256 : (k + 1) * 256]
                              )

                  out_halves = [
                      o_pool.tile(
                          [128, 16 * 256], F32, name=f"out_half_{b}_{i}", tag="out_half"
                      )
                      for i in range(2)
                  ]
                  for wp in range(16):
                      out = out_halves[wp // 8]
                      wo = wp % 8  # wp within half
                      # ---- T1: Z2[(rr,c), (w2,u,kc')] for w pair ----
                      ps = t1_psum.tile([128, 512], F32R)
                      for k in range(2):
                          w = wp * 2 + k
                          for u in range(2):
                              f0 = (w * 8 + u * 4) * 32
                              nc.tensor.transpose(
                                  ps[:, k * 256 + u * 128 : k * 256 + (u + 1) * 128],
                                  bb[:, f0 : f0 + 128],
                                  id_t[:],
                              )
                      z2 = z2_pool.tile([128, 512], F32R)
                      nc.scalar.copy(z2[:], ps[:])

                      # ---- stage B ----
                      psb = b_psum.tile([128, 512], F32)
                      nc.tensor.matmul(psb[:], m32_r, z2[:])
                      nc.vector.tensor_copy(out[:, wo * 512 : (wo + 1) * 512], psb[:])

                      # ---- DMA out after each half: [(rr',c'), (w,u,kc')] -> y ----
                      if wo == 7:
                          h = wp // 8
                          yb = y_d[r0 : r0 + BLOCK_ROWS, :].rearrange(
                              "(w u rr) (cp kc) -> rr cp w u kc", w=32, u=2, rr=4, cp=32
                          )
                          ob = out[:].rearrange("p (w u kc) -> p w u kc", w=16, u=2)
                          w0 = h * 16
                          nc.sync.dma_start(
                              yb[:, :, w0 : w0 + 8], ob[:, 0:8]
                          )
                          nc.scalar.dma_start(
                              yb[:, :, w0 + 8 : w0 + 16], ob[:, 8:16]
                          )

    nc.compile()
    return nc


_CACHE = {}


def _get_program():
    if "nc" not in _CACHE:
        _CACHE["nc"] = build_program()
    return _CACHE["nc"]


def make_in_maps(x: np.ndarray) -> list:
    x_flat = np.ascontiguousarray(x.reshape(16384, DIM), dtype=np.float32)
    h128, m32, ident = _constants()
    return [
        {
            "x": x_flat[i * ROWS_PER_CORE : (i + 1) * ROWS_PER_CORE],
            "h128": h128,
            "m32": m32,
            "ident": ident,
        }
        for i in range(N_CORES)
    ]


def assemble_output(res: dict) -> np.ndarray:
    return res["y"].reshape(4, 4096, 4096)


def kernel(x: np.ndarray, _trace: bool = False, _trace_kwargs=None) -> np.ndarray:
    assert x.shape == (4, 4096, 4096), x.shape
    x_flat = np.ascontiguousarray(x.reshape(16384, DIM), dtype=np.float32)
    h128, m32, ident = _constants()

    in_maps = []
    for i in range(N_CORES):
        in_maps.append(
            {
                "x": x_flat[i * ROWS_PER_CORE : (i + 1) * ROWS_PER_CORE],
                "h128": h128,
                "m32": m32,
                "ident": ident,
            }
        )

    nc = _get_program()
    res = bass_utils.run_bass_kernel_spmd(
        nc,
        in_maps,
        core_ids=list(range(N_CORES)),
        trace=_trace,
        **(_trace_kwargs or {}),
    )
    outs = [res.results[i]["y"] for i in range(N_CORES)]
    y = np.concatenate(outs, axis=0).reshape(4, 4096, 4096)
    if _trace:
        _CACHE["last_result"] = res
    return y



# revision 11
# speedup vs baseline: 7.2193x; 1.3467x over previous
"""FWHT (normalized Walsh-Hadamard transform) over the last dim of x[4,4096,4096].

Rows are independent -> shard 16384 rows across 8 NeuronCores (2048 each).
Per row, H_4096 = H_32 (x) H_128 (Sylvester Kronecker factorization); with
the row viewed as X[c, k] (c in [0,32), k in [0,128)):

    y[c'*128+k'] = (1/64) * sum_{c,k} H32[c',c] H128[k',k] X[c,k]

All storage and PE streams are bf16 (inputs cast on host; |rel err| ~4e-3,
well inside the 2e-2 gate); PSUM accumulation is fp32.  Per 256-row block:

  T0:  PE transposes x[r, (c,k)] tiles          -> Z[k | (c, r)]      bf16
  A :  matmul, stationary H128/8, moving Z      -> PSUM fp32
       ACT copies (cast)                        -> B[k' | (c, r)]     bf16
  T1:  PE transposes strided (c,rr) slices of B -> Z2[(c,rr) | (g,k')] bf16
  B :  matmul, stationary kron(H32,I4)/8        -> PSUM fp32
       copies                                   -> OUT[(c',rr') | (g,k')] f32
  DMA out with 512B-contiguous k' runs.
"""

import numpy as np

try:
    import concourse.bass as bass  # noqa: F401
except ImportError:
    import sys

    sys.path.insert(0, "/opt/trn_rl_repo")

from concourse import bacc, bass, bass_utils, tile
from concourse import mybir

F32 = mybir.dt.float32
BF16 = mybir.dt.bfloat16

N_CORES = 8
ROWS_PER_CORE = 2048
DIM = 4096
N_BLOCKS = 8       # blocks of 256 rows per core
BLOCK_ROWS = 256


def _hadamard(n: int) -> np.ndarray:
    h = np.array([[1.0]], dtype=np.float64)
    while h.shape[0] < n:
        h = np.block([[h, h], [h, -h]])
    return h


def _constants():
    import ml_dtypes

    bf = ml_dtypes.bfloat16
    h128 = (_hadamard(128) * 0.125).astype(bf)
    # stage-B stationary: contraction index p=(c*4+rr), output q=(c'*4+rr')
    k32 = (np.kron(_hadamard(32), np.eye(4)) * 0.125).astype(bf)
    ident = np.eye(128).astype(bf)
    return h128, k32, ident


def build_program(repeat: int = 1, hw_loop: bool = False):
    nc = bacc.Bacc(
        "TRN2",
        target_bir_lowering=False,
        debug=False,
        enable_asserts=False,
    )

    x_d = nc.dram_tensor("x", [ROWS_PER_CORE, DIM], BF16, kind="ExternalInput").ap()
    h128_d = nc.dram_tensor("h128", [128, 128], BF16, kind="ExternalInput").ap()
    k32_d = nc.dram_tensor("k32", [128, 128], BF16, kind="ExternalInput").ap()
    id_d = nc.dram_tensor("ident", [128, 128], BF16, kind="ExternalInput").ap()
    y_d = nc.dram_tensor("y", [ROWS_PER_CORE, DIM], F32, kind="ExternalOutput").ap()

    with tile.TileContext(nc) as tc:
        with (
            tc.tile_pool(name="const", bufs=1) as const_pool,
            tc.tile_pool(name="xin", bufs=4) as x_pool,
            tc.tile_pool(name="zbuf", bufs=2) as z_pool,
            tc.tile_pool(name="bbuf", bufs=2) as b_pool,
            tc.tile_pool(name="z2buf", bufs=3) as z2_pool,
            tc.tile_pool(name="obuf", bufs=2) as o_pool,
            tc.tile_pool(name="ps_t0", bufs=2, space="PSUM") as t0_psum,
            tc.tile_pool(name="ps_a", bufs=2, space="PSUM") as a_psum,
            tc.tile_pool(name="ps_t1", bufs=2, space="PSUM") as t1_psum,
            tc.tile_pool(name="ps_b", bufs=2, space="PSUM") as b_psum,
        ):
            h128_t = const_pool.tile([128, 128], BF16)
            k32_t = const_pool.tile([128, 128], BF16)
            id_t = const_pool.tile([128, 128], BF16)
            nc.sync.dma_start(h128_t[:], h128_d)
            nc.sync.dma_start(k32_t[:], k32_d)
            nc.sync.dma_start(id_t[:], id_d)

            import contextlib

            loop_ctx = (
                tc.For_i(0, repeat) if hw_loop and repeat > 1
                else contextlib.nullcontext()
            )
            with loop_ctx:
              for b in range(N_BLOCKS * (1 if hw_loop else repeat)):
                  r0 = (b % N_BLOCKS) * BLOCK_ROWS

                  # ---- load 2 x [128, 4096] bf16 row subtiles ----
                  xt = []
                  for rs in range(2):
                      t = x_pool.tile([128, DIM], BF16)
                      eng = nc.sync if rs == 0 else nc.scalar
                      eng.dma_start(
                          t[:], x_d[r0 + rs * 128 : r0 + (rs + 1) * 128, :]
                      )
                      xt.append(t)

                  # ---- T0: Z[k, g*128 + c*4 + rr]  (r = g*4 + rr) ----
                  # the (c,rr)-interleave scatter happens here, in bf16 at 2x,
                  # so stage-A output is contiguous and T1 reads 128-runs
                  z = z_pool.tile([128, 32 * 256], BF16)
                  zr = z[:].rearrange("p (g c rr) -> p c g rr", g=64, c=32, rr=4)
                  for cp in range(8):
                      ps = t0_psum.tile([128, 1024], BF16)
                      for j in range(4):
                          c = cp * 4 + j
                          for rs in range(2):
                              nc.tensor.transpose(
                                  ps[:, (j * 2 + rs) * 128 : (j * 2 + rs + 1) * 128],
                                  xt[rs][:, c * 128 : (c + 1) * 128],
                                  id_t[:],
                              )
                      psr = ps[:].rearrange("p (c g rr) -> p c g rr", c=4, g=64, rr=4)
                      nc.vector.tensor_copy(zr[:, cp * 4 : (cp + 1) * 4], psr)

                  # ---- stage A: B[k', g*128 + c*4 + rr] (contiguous; cast bf16) ----
                  bb = b_pool.tile([128, 32 * 256], BF16)
                  for m in range(16):
                      ps = a_psum.tile([128, 512], F32)
                      nc.tensor.matmul(
                          ps[:], h128_t[:], z[:, m * 512 : (m + 1) * 512]
                      )
                      nc.scalar.copy(bb[:, m * 512 : (m + 1) * 512], ps[:])

                  # ---- T1 + stage B, 8 row-groups (g = 4 rows) at a time ----
                  out_t = o_pool.tile([128, 64 * 128], F32)
                  for t in range(8):
                      ps = t1_psum.tile([128, 1024], BF16)
                      for j in range(8):
                          g = t * 8 + j
                          nc.tensor.transpose(
                              ps[:, j * 128 : (j + 1) * 128],
                              bb[:, g * 128 : (g + 1) * 128],
                              id_t[:],
                          )
                      z2 = z2_pool.tile([128, 1024], BF16)
                      nc.vector.tensor_copy(z2[:], ps[:])

                      for h in range(2):
                          psb = b_psum.tile([128, 512], F32)
                          nc.tensor.matmul(
                              psb[:], k32_t[:], z2[:, h * 512 : (h + 1) * 512]
                          )
                          # fp32 PSUM reads run 1x on both engines; ACT (1.2
                          # GHz) takes 10 of 16, DVE picks up the rest
                          dst = out_t[:, t * 1024 + h * 512 : t * 1024 + (h + 1) * 512]
                          if (t * 2 + h) % 3 == 0:
                              nc.vector.tensor_copy(dst, psb[:])
                          else:
                              nc.scalar.copy(dst, psb[:])

                  # ---- DMA out: y[r0 + g*4 + rr', c'*128 + k'] ----
                  yb = y_d[r0 : r0 + BLOCK_ROWS, :].rearrange(
                      "(g rr) (c k) -> rr c g k", g=64, rr=4, c=32, k=128
                  )
                  ob = out_t[:].rearrange("(c rr) (g k) -> rr c g k", rr=4, g=64)
                  for rr in range(4):
                      eng = nc.sync if rr % 2 == 0 else nc.scalar
                      eng.dma_start(yb[rr, :, 0:32], ob[rr, :, 0:32])
                      eng.dma_start(yb[rr, :, 32:64], ob[rr, :, 32:64])

    nc.compile()
    return nc


_CACHE = {}


def _get_program():
    if "nc" not in _CACHE:
        _CACHE["nc"] = build_program()
    return _CACHE["nc"]


def make_in_maps(x: np.ndarray) -> list:
    import ml_dtypes

    x_flat = np.ascontiguousarray(
        x.reshape(16384, DIM).astype(ml_dtypes.bfloat16)
    )
    h128, k32, ident = _constants()
    return [
        {
            "x": x_flat[i * ROWS_PER_CORE : (i + 1) * ROWS_PER_CORE],
            "h128": h128,
            "k32": k32,
            "ident": ident,
        }
        for i in range(N_CORES)
    ]


def assemble_output(res: dict) -> np.ndarray:
    return np.ascontiguousarray(res["y"].reshape(4, 4096, 4096))


def kernel(x: np.ndarray, _trace: bool = False, _trace_kwargs=None) -> np.ndarray:
    assert x.shape == (4, 4096, 4096), x.shape
    in_maps = make_in_maps(x)

    nc = _get_program()
    res = bass_utils.run_bass_kernel_spmd(
        nc,
        in_maps,
        core_ids=list(range(N_CORES)),
        trace=_trace,
        **(_trace_kwargs or {}),
    )
    outs = [res.results[i]["y"] for i in range(N_CORES)]
    y = np.concatenate(outs, axis=0).reshape(4, 4096, 4096)
    if _trace:
        _CACHE["last_result"] = res
    return y


# revision 14
# speedup vs baseline: 108.5481x; 15.0358x over previous
"""FWHT (normalized Walsh-Hadamard transform) over the last dim of x[4,4096,4096].

Rows are independent -> shard 16384 rows across 8 NeuronCores (2048 each).
Per row, H_4096 = H_32 (x) H_128 (Sylvester Kronecker factorization); with
the row viewed as X[c, k] (c in [0,32), k in [0,128)):

    y[c'*128+k'] = (1/64) * sum_{c,k} H32[c',c] H128[k',k] X[c,k]

All storage and PE streams are bf16 (inputs cast on host; rel err ~2e-3,
inside the 2e-2 gate); PSUM accumulation is fp32.  Work is organized in
16 half-blocks of 128 rows, software-pipelined so that every engine
(PE / DVE / ACT / DMA) stays busy:

  T0 :  PE transposes x[r, (c,k)] tiles  -> Z[k | (g,c,rr)]   bf16  (DVE drain,
        scattered so stage A output is g-contiguous; r = g*4+rr)
  A  :  matmul H128/8 x Z                -> B[k' | (g,c,rr)]  (ACT drain, cast)
  T1 :  PE transposes B[:, g*128:+128]   -> Z2[(c,rr) | k']   bf16  (DVE drain)
  B  :  matmul kron(H32,I4)/8 x Z2       -> OUT[(c',rr') | (g,k')] f32
  out:  4 DMAs per half (one per rr), 512B-contiguous k' runs.

Emission interleaves, per half-block: [A,A,T1+B] x4 with the NEXT half's
T0 tiles injected each iteration, so DVE (bf16 drains) and ACT (fp32
drains) overlap instead of alternating in phases.
"""

import numpy as np

try:
    import concourse.bass as bass  # noqa: F401
except ImportError:
    import sys

    sys.path.insert(0, "/opt/trn_rl_repo")

from concourse import bacc, bass, bass_utils, tile
from concourse import mybir

F32 = mybir.dt.float32
BF16 = mybir.dt.bfloat16

N_CORES = 8
ROWS_PER_CORE = 2048
DIM = 4096
N_BLOCKS = 8
BLOCK_ROWS = 256
N_HALVES = 16  # 128-row halves per pass


def _hadamard(n: int) -> np.ndarray:
    h = np.array([[1.0]], dtype=np.float64)
    while h.shape[0] < n:
        h = np.block([[h, h], [h, -h]])
    return h


def _constants():
    import ml_dtypes

    bf = ml_dtypes.bfloat16
    h128 = (_hadamard(128) * 0.125).astype(bf)
    # stage-B stationary: contraction index p=(c*4+rr), output q=(c'*4+rr')
    k32 = (np.kron(_hadamard(32), np.eye(4)) * 0.125).astype(bf)
    ident = np.eye(128).astype(bf)
    return h128, k32, ident


def build_program(repeat: int = 1, hw_loop: bool = False):
    nc = bacc.Bacc(
        "TRN2",
        target_bir_lowering=False,
        debug=False,
        enable_asserts=False,
    )

    x_d = nc.dram_tensor("x", [ROWS_PER_CORE, DIM], BF16, kind="ExternalInput").ap()
    h128_d = nc.dram_tensor("h128", [128, 128], BF16, kind="ExternalInput").ap()
    k32_d = nc.dram_tensor("k32", [128, 128], BF16, kind="ExternalInput").ap()
    id_d = nc.dram_tensor("ident", [128, 128], BF16, kind="ExternalInput").ap()
    y_d = nc.dram_tensor("y", [ROWS_PER_CORE, DIM], F32, kind="ExternalOutput").ap()

    with tile.TileContext(nc) as tc:
        with (
            tc.tile_pool(name="const", bufs=1) as const_pool,
            tc.tile_pool(name="xin", bufs=6) as x_pool,
            tc.tile_pool(name="zbuf", bufs=3) as z_pool,
            tc.tile_pool(name="bbuf", bufs=2) as b_pool,
            tc.tile_pool(name="z2buf", bufs=3) as z2_pool,
            tc.tile_pool(name="obuf", bufs=3) as o_pool,
            tc.tile_pool(name="ps_t0", bufs=2, space="PSUM") as t0_psum,
            tc.tile_pool(name="ps_a", bufs=2, space="PSUM") as a_psum,
            tc.tile_pool(name="ps_t1", bufs=2, space="PSUM") as t1_psum,
            tc.tile_pool(name="ps_b", bufs=2, space="PSUM") as b_psum,
        ):
            h128_t = const_pool.tile([128, 128], BF16)
            k32_t = const_pool.tile([128, 128], BF16)
            id_t = const_pool.tile([128, 128], BF16)
            nc.sync.dma_start(h128_t[:], h128_d)
            nc.sync.dma_start(k32_t[:], k32_d)
            nc.sync.dma_start(id_t[:], id_d)

            xts = {}  # half -> x tile [128, 4096]
            z_tiles = {}  # half -> Z tile
            bb_tiles = {}
            out_tiles = {}

            def emit_in(h):
                b, rs = divmod(h, 2)
                t = x_pool.tile([128, DIM], BF16, name=f"x_{h}", tag="x")
                eng = nc.sync if rs == 0 else nc.scalar
                r0 = (b % N_BLOCKS) * BLOCK_ROWS + rs * 128
                eng.dma_start(t[:], x_d[r0 : r0 + 128, :])
                xts[h] = t

            def emit_t0(h, j):
                # j-th T0 psum tile = 8 transposes, c in [8j, 8j+8)
                if j == 0:
                    z_tiles[h] = z_pool.tile([128, DIM], BF16, name=f"z_{h}", tag="z")
                z = z_tiles[h]
                xt = xts.pop(h) if j == 3 else xts[h]
                ps = t0_psum.tile([128, 1024], BF16, name=f"t0ps_{h}_{j}", tag="t0ps")
                for i in range(8):
                    c = j * 8 + i
                    nc.tensor.transpose(
                        ps[:, i * 128 : (i + 1) * 128],
                        xt[:, c * 128 : (c + 1) * 128],
                        id_t[:],
                    )
                zr = z[:].rearrange("p (g c rr) -> p c g rr", g=32, c=32, rr=4)
                psr = ps[:].rearrange("p (c g rr) -> p c g rr", c=8, g=32, rr=4)
                nc.vector.tensor_copy(zr[:, j * 8 : (j + 1) * 8], psr)

            def emit_a(h, m):
                if m == 0:
                    bb_tiles[h] = b_pool.tile([128, DIM], BF16, name=f"bb_{h}", tag="bb")
                z = z_tiles[h]
                bb = bb_tiles[h]
                ps = a_psum.tile([128, 512], F32, name=f"aps_{h}_{m}", tag="aps")
                nc.tensor.matmul(ps[:], h128_t[:], z[:, m * 512 : (m + 1) * 512])
                nc.scalar.copy(bb[:, m * 512 : (m + 1) * 512], ps[:])
                if m == 7:
                    del z_tiles[h]

            def emit_t1b(h, t):
                if t == 0:
                    out_tiles[h] = o_pool.tile([128, DIM], F32, name=f"out_{h}", tag="out")
                bb = bb_tiles[h]
                out = out_tiles[h]
                ps = t1_psum.tile([128, 1024], BF16, name=f"t1ps_{h}_{t}", tag="t1ps")
                for j in range(8):
                    g = t * 8 + j
                    nc.tensor.transpose(
                        ps[:, j * 128 : (j + 1) * 128],
                        bb[:, g * 128 : (g + 1) * 128],
                        id_t[:],
                    )
                z2 = z2_pool.tile([128, 1024], BF16, name=f"z2_{h}_{t}", tag="z2")
                nc.vector.tensor_copy(z2[:], ps[:])
                # next half's T0 tile here: PE fills the z2-drain latency gap
                if h + 1 < N_HALVES or _wrap:
                    emit_t0((h + 1) % N_HALVES, t)
                for hh in range(2):
                    psb = b_psum.tile([128, 512], F32, name=f"bps_{h}_{t}_{hh}", tag="bps")
                    nc.tensor.matmul(
                        psb[:], k32_t[:], z2[:, hh * 512 : (hh + 1) * 512]
                    )
                    dst = out[:, t * 1024 + hh * 512 : t * 1024 + (hh + 1) * 512]
                    # 3 of 8 fp32 B-drains go to DVE, the rest to ACT
                    if (t * 2 + hh) % 8 < 3:
                        nc.vector.tensor_copy(dst, psb[:])
                    else:
                        nc.scalar.copy(dst, psb[:])
                if t == 3:
                    del bb_tiles[h]

            def emit_out(h):
                b, rs = divmod(h, 2)
                out = out_tiles.pop(h)
                r0 = (b % N_BLOCKS) * BLOCK_ROWS + rs * 128
                yv = y_d[r0 : r0 + 128, :].rearrange(
                    "(g rr) (c k) -> rr c g k", g=32, rr=4, c=32, k=128
                )
                ob = out[:].rearrange("(c rr) (g k) -> rr c g k", rr=4, g=32)
                for rr in range(4):
                    eng = nc.sync if (rr + rs) % 2 == 0 else nc.scalar
                    eng.dma_start(yv[rr], ob[rr])

            def emit_pass():
                emit_in(0)
                emit_in(1)
                for j in range(4):
                    emit_t0(0, j)
                for H in range(N_HALVES):
                    if H + 2 < N_HALVES:
                        emit_in(H + 2)
                    emit_a(H, 0)
                    emit_a(H, 1)
                    for t in range(4):
                        if t < 3:
                            emit_a(H, 2 * t + 2)
                            emit_a(H, 2 * t + 3)
                        emit_t1b(H, t)
                        if H + 1 < N_HALVES:
                            emit_t0(H + 1, t)
                    emit_out(H)

            import contextlib

            loop_ctx = (
                tc.For_i(0, repeat) if hw_loop and repeat > 1
                else contextlib.nullcontext()
            )
            with loop_ctx:
                for _ in range(1 if hw_loop else repeat):
                    emit_pass()

    nc.compile()
    return nc


_CACHE = {}


def _get_program():
    if "nc" not in _CACHE:
        _CACHE["nc"] = build_program()
    return _CACHE["nc"]


def make_in_maps(x: np.ndarray) -> list:
    import ml_dtypes

    x_flat = np.ascontiguousarray(
        x.reshape(16384, DIM).astype(ml_dtypes.bfloat16)
    )
    h128, k32, ident = _constants()
    return [
        {
            "x": x_flat[i * ROWS_PER_CORE : (i + 1) * ROWS_PER_CORE],
            "h128": h128,
            "k32": k32,
            "ident": ident,
        }
        for i in range(N_CORES)
    ]


def assemble_output(res: dict) -> np.ndarray:
    return np.ascontiguousarray(res["y"].reshape(4, 4096, 4096))


def kernel(x: np.ndarray, _trace: bool = False, _trace_kwargs=None) -> np.ndarray:
    assert x.shape == (4, 4096, 4096), x.shape
    in_maps = make_in_maps(x)

    nc = _get_program()
    res = bass_utils.run_bass_kernel_spmd(
        nc,
        in_maps,
        core_ids=list(range(N_CORES)),
        trace=_trace,
        **(_trace_kwargs or {}),
    )
    outs = [res.results[i]["y"] for i in range(N_CORES)]
    y = np.concatenate(outs, axis=0).reshape(4, 4096, 4096)
    if _trace:
        _CACHE["last_result"] = res
    return y
